# revision 35
# baseline (speedup 1.0000x reference)
"""Trainium2 Bass kernel for nn_HardQuadTripletSOSRLoss.

Sharding: 8 cores = 2 batches x 4 HW-shards (4096 grid cells each).

Device (per core): dsim candidate extraction only.
  - inputs host-quantized: kp1_desc^T as bf16 stationary, desc2 shard as
    fp8(e4m3) moving operand (halves DMA; matmul runs at bf16 speed)
  - a few garbage warm-up matmuls release the PE HAM clock throttle before
    the real data lands; rhs arrives as two 2048-cell pieces on separate
    DMA queues (scalar HWDGE + gpsimd SWDGE)
  - PE: scores = kp1_desc[b] @ desc2f[b, shard]^T, 512-col matmuls into
    [128, 1024] PSUM tiles (2 banks x 4 slots)
  - per 4096-cell row-tile: DVE max8 over a 1024-wide direct f32 chunk;
    ACT converts the other 3072 cells to bf16 in SBUF and DVE runs a
    2x-mode pairwise-max tree + final max8 over them
  - exports top-8 VALUES per chunk only (no indices)

Host: bilinear sampling, grid geometry, masks, k_sim/w_sim top-8 (512-wide,
cheap), distributed top-k merge with mask patching by value-match against
host-requantized scores (remove matched raw values, insert exact adjusted
values) + a per-chunk certificate; rows whose certificate fails are
recomputed exactly, and a whole-row numerics guard falls back to exact
host evaluation if the device quantization model ever stops replicating.
"""

import numpy as np
import ml_dtypes

import concourse.bass as bass
import concourse.mybir as mybir
import concourse.tile as tile
from concourse import bacc
from concourse.bass_utils import run_bass_kernel_spmd

# ---- problem constants (hardcoded per contract) ----
B, N, C, H, W = 2, 512, 128, 128, 128
HW = H * W
GS = 8
NUM_NEG = 16
SOS_NEG = 8
MARGIN = 1.0
NSHARD = 4
SHW = HW // NSHARD          # 4096 cells per shard
RT = N // 128               # 4 row tiles

F32 = mybir.dt.float32
BF16 = mybir.dt.bfloat16
F8 = mybir.dt.float8e4
BF = ml_dtypes.bfloat16
NP8 = ml_dtypes.float8_e4m3

SCAN_MODE = "split"         # "direct" | "split"
N_WARM = 4                  # dummy matmuls to release the PE HAM throttle

# per-row-tile chunk layout within one 4096-cell shard:
#   (start_cell, end_cell, kind)  kind: "f32" (exact top8) | "bf16" (tree)
if SCAN_MODE == "direct":
    CHUNKS = [(0, 2048, "f32"), (2048, 4096, "f32")]
else:
    CHUNKS = [(0, 1024, "f32"), (1024, 4096, "bf16")]
NF32 = sum(1 for c in CHUNKS if c[2] == "f32")
NBF = sum(1 for c in CHUNKS if c[2] == "bf16")

# rhs DMA pieces: (start_cell, n_cells, issue_engine); 2048-cell pieces keep
# 2KB descriptor rows (fp8) for decent per-queue DMA bandwidth
RHS_PIECES = [
    (0, 2048, "scalar"),
    (2048, 2048, "gpsimd"),
]

_NC_CACHE = {}
LAST_RESULTS = None  # BassKernelResults of most recent device run (for test.py)


def _build_nc():
    nc = bacc.Bacc("TRN2", target_bir_lowering=False, debug=False, num_devices=8)

    lhsT = nc.dram_tensor("lhsT", [C, N], BF16, kind="ExternalInput")
    rhs_dram = [
        nc.dram_tensor(f"rhs{i}", [C, npc], F8, kind="ExternalInput")
        for i, (c0, npc, eng) in enumerate(RHS_PIECES)
    ]
    nch = len(CHUNKS)
    cand = nc.dram_tensor("cand", [RT, 128, nch * 8], F32, kind="ExternalOutput")

    with tile.TileContext(nc) as tc:
        with (
            tc.tile_pool(name="const", bufs=1) as cpool,
            tc.tile_pool(name="cnd", bufs=2) as cndpool,
            tc.tile_pool(name="tree", bufs=2) as trpool,
            tc.tile_pool(name="psum", bufs=4, space="PSUM") as pspool,
        ):
            # PE warm-up: garbage matmuls release the HAM clock throttle
            # (~3.4us of sustained activity) before the real data lands.
            warm_w = cpool.tile([C, 128], BF16, tag="warmw")
            warm_x = cpool.tile([C, 512], F8, tag="warmx")
            nc.vector.memset(warm_w[:], 0.0)
            nc.vector.memset(warm_x[:], 0.0)
            if N_WARM:
                wp = pspool.tile([128, 1024], F32, tag="ps")
                for _ in range(N_WARM):
                    nc.tensor.matmul(wp[:, 0:512], warm_w[:], warm_x[:],
                                     start=True, stop=True)

            lhsT_sb = cpool.tile([C, N], BF16, tag="lhsT")
            nc.sync.dma_start(lhsT_sb[:], lhsT[:, :])
            rhs_sb = []
            for i, (c0, npc, eng) in enumerate(RHS_PIECES):
                t = cpool.tile([C, npc], F8, tag=f"rhs{i}")
                getattr(nc, eng).dma_start(t[:], rhs_dram[i][:, :])
                rhs_sb.append(t)

            def weights(t):
                return lhsT_sb[:, t * 128:(t + 1) * 128]

            def mm(ps_slice, t, cell0):
                # 512-col matmul: scores for shard cells [cell0, cell0+512)
                for i, (c0, npc, eng) in enumerate(RHS_PIECES):
                    if c0 <= cell0 < c0 + npc:
                        piece, col = rhs_sb[i], cell0 - c0
                        break
                nc.tensor.matmul(ps_slice, weights(t), piece[:, col:col + 512],
                                 start=True, stop=True)

            if SCAN_MODE == "direct":
                for t in range(RT):
                    cn = cndpool.tile([128, nch * 8], F32, tag="cn")
                    for ci in range(2):
                        ps = pspool.tile([128, 2048], F32, tag="ps")
                        for k in range(4):
                            mm(ps[:, k * 512:(k + 1) * 512], t,
                               ci * 2048 + k * 512)
                        nc.vector.max(cn[:, ci * 8:(ci + 1) * 8], ps[:])
                    nc.sync.dma_start(cand[t], cn[:])
            else:
                # per row-tile: direct f32 max8 over cells [0:1024) plus a
                # bf16 max tree (ACT converts, DVE 2x TT-max) over [1024:4096)
                for t in range(RT):
                    cn = cndpool.tile([128, nch * 8], F32, tag="cn")
                    ps = {}
                    cv = {}
                    for q in (1, 0, 2, 3):
                        p = pspool.tile([128, 1024], F32, tag="ps")
                        mm(p[:, 0:512], t, q * 1024)
                        mm(p[:, 512:1024], t, q * 1024 + 512)
                        ps[q] = p
                        if q:
                            c = trpool.tile([128, 1024], BF16, tag=f"c{q}")
                            nc.scalar.copy(c[:], p[:])
                            cv[q] = c
                    # direct chunk: cells [0, 1024)
                    nc.vector.max(cn[:, 0:8], ps[0][:])
                    m1 = trpool.tile([128, 1024], BF16, tag="m1")
                    nc.vector.tensor_max(m1[:], cv[1][:], cv[2][:])
                    m2 = trpool.tile([128, 1024], BF16, tag="m2")
                    nc.vector.tensor_max(m2[:], m1[:], cv[3][:])
                    m3 = trpool.tile([128, 512], BF16, tag="m3")
                    nc.vector.tensor_max(m3[:], m2[:, 0:512], m2[:, 512:1024])
                    # bf16 tree values written into the f32 cn tile
                    nc.vector.max(cn[:, 8:16], m3[:])
                    nc.sync.dma_start(cand[t], cn[:])

    nc.compile()
    return nc


def _get_nc():
    if "nc" not in _NC_CACHE:
        _NC_CACHE["nc"] = _build_nc()
    return _NC_CACHE["nc"]


# ---------------- host-side helpers (all float32, mirror reference) ----------


def _sample_descriptors(desc2, kp):
    """Bilinear sample of desc2 (B,C,H,W) at image-space (y,x) kp, L2-normed."""
    b, c, h, w = desc2.shape
    f = np.float32
    y = np.clip(kp[..., 0] / f(GS) - f(0.5), f(0.0), f(h - 1.0)).astype(f)
    x = np.clip(kp[..., 1] / f(GS) - f(0.5), f(0.0), f(w - 1.0)).astype(f)
    y0 = np.clip(np.floor(y), 0, h - 2).astype(np.int64)
    x0 = np.clip(np.floor(x), 0, w - 2).astype(np.int64)
    wy = (y - y0.astype(f))[..., None]
    wx = (x - x0.astype(f))[..., None]
    dmap = desc2.transpose(0, 2, 3, 1).reshape(b, h * w, c)

    def g(yi, xi):
        idx = yi * w + xi
        return np.take_along_axis(dmap, idx[..., None], axis=1)

    v = (
        g(y0, x0) * (1 - wy) * (1 - wx)
        + g(y0, x0 + 1) * (1 - wy) * wx
        + g(y0 + 1, x0) * wy * (1 - wx)
        + g(y0 + 1, x0 + 1) * wy * wx
    )
    n = np.sqrt(np.sum(v * v, axis=-1, keepdims=True)).astype(f)
    return (v / (n + f(1e-8))).astype(f)


def _nearest4(pts):
    """Flat ids (..., 4) of the 4 nearest grid-cell centers, matching the
    reference's top_k over all HW cells (ties -> lower flat id)."""
    f = np.float32
    y = pts[..., 0]
    x = pts[..., 1]
    cy = np.clip(np.floor(y / f(GS)).astype(np.int64), 0, H - 1)
    cx = np.clip(np.floor(x / f(GS)).astype(np.int64), 0, W - 1)
    by = np.clip(cy - 2, 0, H - 5)
    bx = np.clip(cx - 2, 0, W - 5)
    offs = np.arange(5, dtype=np.int64)
    iy = by[..., None] + offs          # (..., 5)
    ix = bx[..., None] + offs
    cyc = (f(GS) * iy + f(GS / 2.0)).astype(f)
    cxc = (f(GS) * ix + f(GS / 2.0)).astype(f)
    dy = y[..., None] - cyc
    dx = x[..., None] - cxc
    d2 = (dy * dy)[..., :, None] + (dx * dx)[..., None, :]   # (..., 5, 5)
    ids = iy[..., :, None] * W + ix[..., None, :]
    d2 = d2.reshape(d2.shape[:-2] + (25,))
    ids = ids.reshape(ids.shape[:-2] + (25,))
    # candidates are flat-id ascending, so a stable sort on d2 reproduces
    # top_k's lower-index tie-break
    order = np.argsort(d2, axis=-1, kind="stable")[..., :4]
    return np.take_along_axis(ids, order, axis=-1)


def _warp(p, Hm):
    f = np.float32
    xy = p[..., ::-1]
    ph = np.concatenate([xy, np.ones_like(xy[..., :1])], axis=-1)
    wp = np.einsum("bij,bmj->bmi", Hm, ph).astype(f)
    wp = wp[..., :2] / (wp[..., 2:3] + f(1e-8))
    return wp[..., ::-1].astype(f)


def _centers(ids):
    f = np.float32
    yy = (ids // W).astype(f) * f(GS) + f(GS / 2.0)
    xx = (ids % W).astype(f) * f(GS) + f(GS / 2.0)
    return np.stack([yy, xx], axis=-1)


def _smallest8_ids(x):
    """Indices of the 8 smallest values per row, lax.top_k tie semantics
    (ties -> lower index). x: (N, M) -> (N, 8)."""
    return np.argsort(x, axis=-1, kind="stable")[:, :SOS_NEG]


def kernel(kp1, w_kp1, kp1_desc, desc2, homo12):
    global LAST_RESULTS
    import os

    f = np.float32
    kp1 = np.asarray(kp1, f)
    w_kp1 = np.asarray(w_kp1, f)
    kp1_desc = np.asarray(kp1_desc, f)
    desc2 = np.asarray(desc2, f)
    homo12 = np.asarray(homo12, f)

    # ---------------- host geometry / small tensors ----------------
    w_kp1_desc = _sample_descriptors(desc2, w_kp1)                  # (B,N,C)
    pos = f(2.0) - f(2.0) * np.einsum("bnc,bnc->bn", kp1_desc, w_kp1_desc)

    cell4 = _nearest4(kp1)                                          # (B,N,4)
    kp1_cells = _centers(cell4.reshape(B, 4 * N))                   # (B,4N,2)
    warped = _warp(kp1_cells, homo12)                               # (B,4N,2)
    wcc = _nearest4(warped)                                         # (B,4N,4)
    ids16 = wcc.reshape(B, N, 16)                                   # neigh cells
    cell4_w = _nearest4(w_kp1)                                      # (B,N,4)

    # kp1_mask[n,n'] = #coinciding cells between cell4[n] and cell4[n']
    eqk = cell4[:, :, :, None, None] == cell4[:, None, None, :, :]
    kp1_mask = eqk.sum(axis=(2, 4)).astype(f)                       # (B,N,N)
    # w_kp1_mask[n,n'] = #coincidences between ids16[n] and cell4_w[n']
    eqw = ids16[:, :, :, None, None] == cell4_w[:, None, None, :, :]
    w_kp1_mask = eqw.sum(axis=(2, 4)).astype(f)                     # (B,N,N)

    # ---------------- device run ----------------
    nc = _get_nc()
    desc2_flat = np.ascontiguousarray(desc2.reshape(B, C, HW))
    lhsT_q = [np.ascontiguousarray(kp1_desc[b].T.astype(BF)) for b in range(B)]
    in_maps = []
    for b in range(B):
        for s in range(NSHARD):
            m = {"lhsT": lhsT_q[b]}
            for i, (c0, npc, eng) in enumerate(RHS_PIECES):
                m[f"rhs{i}"] = np.ascontiguousarray(
                    desc2_flat[b][:, s * SHW + c0:s * SHW + c0 + npc].astype(NP8)
                )
            in_maps.append(m)
    want_trace = bool(int(os.environ.get("KT_TRACE", "0")))
    try:
        res = run_bass_kernel_spmd(
            nc, in_maps, core_ids=list(range(8)), trace=want_trace
        )
    except ModuleNotFoundError:
        res = run_bass_kernel_spmd(nc, in_maps, core_ids=list(range(8)), trace=False)
    LAST_RESULTS = res
    results = res.results

    # candidate values per row: NSHARD shards x len(CHUNKS) chunks x 8, f32
    nch = len(CHUNKS)
    cand_all = np.empty((B, N, NSHARD, nch, 8), f)
    for ci, (b, s) in enumerate((b, s) for b in range(B) for s in range(NSHARD)):
        cf = results[ci]["cand"]                        # (RT,128,nch*8) f32
        for t in range(RT):
            rows = slice(t * 128, (t + 1) * 128)
            for k in range(nch):
                cand_all[b, rows, s, k, :] = cf[t][:, k * 8:(k + 1) * 8]

    # ---------------- fos: merge per-shard candidates ----------------
    # exact (host) raw scores of masked cells, replicating the device's
    # bf16-weights x e4m3-moving matmul: f32 accumulation over cast operands
    lhq = np.ascontiguousarray(  # (B,N,C) f32 of bf16
        np.stack([lhsT_q[b].T.astype(f) for b in range(B)]))
    dq = desc2_flat.astype(NP8).astype(f)               # (B,C,HW)
    vm16 = np.empty((B, N, 16), f)
    for b in range(B):
        gath = dq[b][:, ids16[b].reshape(-1)].reshape(C, N, 16)
        vm16[b] = np.einsum("nc,cnk->nk", lhq[b], gath)

    # numerics guard: recompute one row on the host and compare each chunk's
    # maximum (the max always survives the device's max tree); if PE numerics
    # don't replicate the host quantization model, fall back to exact host
    # evaluation for every row (correct, just slower on the host)
    g_row = dq[0].T @ lhq[0, 0]                         # (HW,)
    replication_ok = True
    for s in range(NSHARD):
        for k, (c0, c1, kind) in enumerate(CHUNKS):
            seg = g_row[s * SHW + c0:s * SHW + c1]
            if kind == "bf16":
                seg = seg.astype(BF).astype(f)
            if abs(float(seg.max()) - float(cand_all[0, 0, s, k].max())) > 0.02:
                replication_ok = False

    # chunk id (shard, chunk) for every cell
    cell_chunk = np.empty(HW, np.int64)
    chunk_kind = []
    for s in range(NSHARD):
        for k, (c0, c1, kind) in enumerate(CHUNKS):
            cell_chunk[s * SHW + c0: s * SHW + c1] = s * nch + k
            chunk_kind.append(kind)

    flat = cand_all.reshape(B, N, NSHARD * nch, 8)
    chunk_min = flat[..., 7]                            # (B,N,nchunks)
    TOL_F = 1e-3
    TOL_B = 0.033                                       # ~1 ulp bf16 at |x|~4
    neg_scores = np.empty((B, N, NUM_NEG), f)
    if not replication_ok:
        repair = [(b, n) for b in range(B) for n in range(N)]
    else:
        repair = []
    for b in range(B):
        for n in range(N):
            if not replication_ok:
                continue
            vals = flat[b, n].copy()                    # (nchunks, 8)
            alive = np.ones_like(vals, bool)
            uq, cnts = np.unique(ids16[b, n], return_counts=True)
            bad = False
            add = np.empty(len(uq), f)
            for i, (u, cu) in enumerate(zip(uq, cnts)):
                # value of this masked cell under device convention
                j = int(np.argmax(ids16[b, n] == u))
                v = vm16[b, n, j]
                ch = cell_chunk[u]
                kind = chunk_kind[ch]
                vq = f(BF(v)) if kind == "bf16" else v
                add[i] = v - f(2.5) * cu
                tol = TOL_B if kind == "bf16" else TOL_F
                if vq >= chunk_min[b, n, ch] - tol:
                    row = vals[ch]
                    cand_idx = np.where(alive[ch])[0]
                    if len(cand_idx):
                        d = np.abs(row[cand_idx] - vq)
                        jj = int(np.argmin(d))
                        if d[jj] <= tol:
                            alive[ch, cand_idx[jj]] = False
                        elif kind == "f32":
                            bad = True  # should have been exported; wasn't
                    # bf16 chunks: no match => shadowed by tree, accept
            if not bad:
                pool = np.concatenate([vals[alive], add])
                pool.sort()
                top = pool[::-1][:NUM_NEG]
                thr = top[-1]
                # certificate: no chunk may conceal values above thr
                for ch in range(NSHARD * nch):
                    tol = TOL_B if chunk_kind[ch] == "bf16" else TOL_F
                    if chunk_min[b, n, ch] >= thr - tol:
                        bad = True
                        break
            if bad:
                repair.append((b, n))
            else:
                neg_scores[b, n] = top

    if repair:
        hwdesc = desc2_flat.transpose(0, 2, 1)          # (B,HW,C) f32 exact
        for b, n in repair:
            row = hwdesc[b] @ kp1_desc[b, n]            # (HW,)
            np.subtract.at(row, ids16[b, n], f(2.5))
            neg_scores[b, n] = np.sort(row)[::-1][:NUM_NEG]

    neg = f(2.0) - f(2.0) * neg_scores                  # (B,N,16)
    fos = np.mean(
        np.maximum(pos[..., None] - neg + f(MARGIN), f(0.0)) ** 2
    ).astype(f)

    # ---------------- sos (host: 512-wide sims are cheap) ----------------
    k_ids = np.empty((B, N, SOS_NEG), np.int64)
    w_ids = np.empty((B, N, SOS_NEG), np.int64)
    for b in range(B):
        ksim = f(2.0) - f(2.0) * (kp1_desc[b] @ kp1_desc[b].T) \
            + f(5.0) * kp1_mask[b]
        wsim = f(2.0) - f(2.0) * (w_kp1_desc[b] @ w_kp1_desc[b].T) \
            + f(5.0) * w_kp1_mask[b]
        k_ids[b] = _smallest8_ids(ksim)
        w_ids[b] = _smallest8_ids(wsim)

    kd = np.take_along_axis(
        kp1_desc, k_ids.reshape(B, N * SOS_NEG)[:, :, None], axis=1
    ).reshape(B, N, SOS_NEG, C)
    wd = np.take_along_axis(
        w_kp1_desc, w_ids.reshape(B, N * SOS_NEG)[:, :, None], axis=1
    ).reshape(B, N, SOS_NEG, C)
    a = f(2.0) - f(2.0) * np.einsum("bnc,bnkc->bnk", kp1_desc, kd)
    bb = f(2.0) - f(2.0) * np.einsum("bnc,bnkc->bnk", w_kp1_desc, wd)
    sv = (a - bb).astype(f)
    sos = np.mean(np.sqrt(np.sum(sv * sv, axis=-1))).astype(f)

    return np.asarray(fos + sos, dtype=np.float32)


# revision 37
# speedup vs baseline: 1.2044x; 1.2044x over previous
"""Trainium2 Bass kernel for nn_HardQuadTripletSOSRLoss.

Sharding: 8 cores = 2 batches x 4 HW-shards (4096 grid cells each).

Device (per core): dsim candidate extraction only.
  - inputs host-quantized: kp1_desc^T as bf16 stationary, desc2 shard as
    fp8(e4m3) moving operand (halves DMA; matmul runs at bf16 speed)
  - a few garbage warm-up matmuls release the PE HAM clock throttle before
    the real data lands; rhs arrives as two 2048-cell pieces on separate
    DMA queues (scalar HWDGE + gpsimd SWDGE)
  - PE: scores = kp1_desc[b] @ desc2f[b, shard]^T, 512-col matmuls into
    [128, 1024] PSUM tiles (2 banks x 4 slots)
  - per 4096-cell row-tile: DVE max8 over a 1024-wide direct f32 chunk;
    ACT converts the other 3072 cells to bf16 in SBUF and DVE runs a
    2x-mode pairwise-max tree + final max8 over them
  - exports top-8 VALUES per chunk only (no indices)

Host: bilinear sampling, grid geometry, masks, k_sim/w_sim top-8 (512-wide,
cheap), distributed top-k merge with mask patching by value-match against
host-requantized scores (remove matched raw values, insert exact adjusted
values) + a per-chunk certificate; rows whose certificate fails are
recomputed exactly, and a whole-row numerics guard falls back to exact
host evaluation if the device quantization model ever stops replicating.
"""

import numpy as np
import ml_dtypes

import concourse.bass as bass
import concourse.mybir as mybir
import concourse.tile as tile
from concourse import bacc
from concourse.bass_utils import run_bass_kernel_spmd

# ---- problem constants (hardcoded per contract) ----
B, N, C, H, W = 2, 512, 128, 128, 128
HW = H * W
GS = 8
NUM_NEG = 16
SOS_NEG = 8
MARGIN = 1.0
NSHARD = 4
SHW = HW // NSHARD          # 4096 cells per shard
RT = N // 128               # 4 row tiles

F32 = mybir.dt.float32
BF16 = mybir.dt.bfloat16
F8 = mybir.dt.float8e4
BF = ml_dtypes.bfloat16
NP8 = ml_dtypes.float8_e4m3

SCAN_MODE = "split"         # "direct" | "split"
N_WARM = 4                  # dummy matmuls to release the PE HAM throttle

# per-row-tile chunk layout within one 4096-cell shard:
#   (start_cell, end_cell, kind)  kind: "f32" (exact top8) | "bf16" (tree)
if SCAN_MODE == "direct":
    CHUNKS = [(0, 2048, "f32"), (2048, 4096, "f32")]
else:
    # converted chunk first: it consumes the earliest-arriving DMA piece,
    # and each row-tile's critical path ends on the cheap direct max8
    CHUNKS = [(0, 3072, "bf16"), (3072, 4096, "f32")]
NF32 = sum(1 for c in CHUNKS if c[2] == "f32")
NBF = sum(1 for c in CHUNKS if c[2] == "bf16")

# rhs DMA pieces: (start_cell, n_cells, issue_engine); 2048-cell pieces keep
# 2KB descriptor rows (fp8) for decent per-queue DMA bandwidth
RHS_PIECES = [
    (0, 2048, "scalar"),
    (2048, 2048, "gpsimd"),
]

_NC_CACHE = {}
LAST_RESULTS = None  # BassKernelResults of most recent device run (for test.py)


def _build_nc():
    nc = bacc.Bacc("TRN2", target_bir_lowering=False, debug=False, num_devices=8)

    lhsT = nc.dram_tensor("lhsT", [C, N], BF16, kind="ExternalInput")
    rhs_dram = [
        nc.dram_tensor(f"rhs{i}", [C, npc], F8, kind="ExternalInput")
        for i, (c0, npc, eng) in enumerate(RHS_PIECES)
    ]
    nch = len(CHUNKS)
    cand = nc.dram_tensor("cand", [RT, 128, nch * 8], F32, kind="ExternalOutput")

    with tile.TileContext(nc) as tc:
        with (
            tc.tile_pool(name="const", bufs=1) as cpool,
            tc.tile_pool(name="cnd", bufs=2) as cndpool,
            tc.tile_pool(name="tree", bufs=2) as trpool,
            tc.tile_pool(name="psum", bufs=4, space="PSUM") as pspool,
        ):
            # PE warm-up: garbage matmuls release the HAM clock throttle
            # (~3.4us of sustained activity) before the real data lands.
            warm_w = cpool.tile([C, 128], BF16, tag="warmw")
            warm_x = cpool.tile([C, 512], F8, tag="warmx")
            nc.vector.memset(warm_w[:], 0.0)
            nc.vector.memset(warm_x[:], 0.0)
            if N_WARM:
                wp = pspool.tile([128, 1024], F32, tag="ps")
                for _ in range(N_WARM):
                    nc.tensor.matmul(wp[:, 0:512], warm_w[:], warm_x[:],
                                     start=True, stop=True)

            lhsT_sb = cpool.tile([C, N], BF16, tag="lhsT")
            nc.sync.dma_start(lhsT_sb[:], lhsT[:, :])
            rhs_sb = []
            for i, (c0, npc, eng) in enumerate(RHS_PIECES):
                t = cpool.tile([C, npc], F8, tag=f"rhs{i}")
                getattr(nc, eng).dma_start(t[:], rhs_dram[i][:, :])
                rhs_sb.append(t)

            def weights(t):
                return lhsT_sb[:, t * 128:(t + 1) * 128]

            def mm(ps_slice, t, cell0):
                # 512-col matmul: scores for shard cells [cell0, cell0+512)
                for i, (c0, npc, eng) in enumerate(RHS_PIECES):
                    if c0 <= cell0 < c0 + npc:
                        piece, col = rhs_sb[i], cell0 - c0
                        break
                nc.tensor.matmul(ps_slice, weights(t), piece[:, col:col + 512],
                                 start=True, stop=True)

            if SCAN_MODE == "direct":
                for t in range(RT):
                    cn = cndpool.tile([128, nch * 8], F32, tag="cn")
                    for ci in range(2):
                        ps = pspool.tile([128, 2048], F32, tag="ps")
                        for k in range(4):
                            mm(ps[:, k * 512:(k + 1) * 512], t,
                               ci * 2048 + k * 512)
                        nc.vector.max(cn[:, ci * 8:(ci + 1) * 8], ps[:])
                    nc.sync.dma_start(cand[t], cn[:])
            else:
                # per row-tile: bf16 max tree (ACT converts, DVE 2x TT-max)
                # over cells [0:3072) plus a direct f32 max8 over [3072:4096)
                for t in range(RT):
                    cn = cndpool.tile([128, nch * 8], F32, tag="cn")
                    ps = {}
                    cv = {}
                    for q in (0, 1, 2, 3):
                        p = pspool.tile([128, 1024], F32, tag="ps")
                        mm(p[:, 0:512], t, q * 1024)
                        mm(p[:, 512:1024], t, q * 1024 + 512)
                        ps[q] = p
                        if q < 3:
                            c = trpool.tile([128, 1024], BF16, tag=f"c{q}")
                            nc.scalar.copy(c[:], p[:])
                            cv[q] = c
                    m1 = trpool.tile([128, 1024], BF16, tag="m1")
                    nc.vector.tensor_max(m1[:], cv[0][:], cv[1][:])
                    m2 = trpool.tile([128, 1024], BF16, tag="m2")
                    nc.vector.tensor_max(m2[:], m1[:], cv[2][:])
                    m3 = trpool.tile([128, 512], BF16, tag="m3")
                    nc.vector.tensor_max(m3[:], m2[:, 0:512], m2[:, 512:1024])
                    # bf16 tree values written into the f32 cn tile
                    nc.vector.max(cn[:, 0:8], m3[:])
                    # direct chunk: cells [3072, 4096)
                    nc.vector.max(cn[:, 8:16], ps[3][:])
                    nc.sync.dma_start(cand[t], cn[:])

    nc.compile()
    return nc


def _get_nc():
    if "nc" not in _NC_CACHE:
        _NC_CACHE["nc"] = _build_nc()
    return _NC_CACHE["nc"]


# ---------------- host-side helpers (all float32, mirror reference) ----------


def _sample_descriptors(desc2, kp):
    """Bilinear sample of desc2 (B,C,H,W) at image-space (y,x) kp, L2-normed."""
    b, c, h, w = desc2.shape
    f = np.float32
    y = np.clip(kp[..., 0] / f(GS) - f(0.5), f(0.0), f(h - 1.0)).astype(f)
    x = np.clip(kp[..., 1] / f(GS) - f(0.5), f(0.0), f(w - 1.0)).astype(f)
    y0 = np.clip(np.floor(y), 0, h - 2).astype(np.int64)
    x0 = np.clip(np.floor(x), 0, w - 2).astype(np.int64)
    wy = (y - y0.astype(f))[..., None]
    wx = (x - x0.astype(f))[..., None]
    dmap = desc2.transpose(0, 2, 3, 1).reshape(b, h * w, c)

    def g(yi, xi):
        idx = yi * w + xi
        return np.take_along_axis(dmap, idx[..., None], axis=1)

    v = (
        g(y0, x0) * (1 - wy) * (1 - wx)
        + g(y0, x0 + 1) * (1 - wy) * wx
        + g(y0 + 1, x0) * wy * (1 - wx)
        + g(y0 + 1, x0 + 1) * wy * wx
    )
    n = np.sqrt(np.sum(v * v, axis=-1, keepdims=True)).astype(f)
    return (v / (n + f(1e-8))).astype(f)


def _nearest4(pts):
    """Flat ids (..., 4) of the 4 nearest grid-cell centers, matching the
    reference's top_k over all HW cells (ties -> lower flat id)."""
    f = np.float32
    y = pts[..., 0]
    x = pts[..., 1]
    cy = np.clip(np.floor(y / f(GS)).astype(np.int64), 0, H - 1)
    cx = np.clip(np.floor(x / f(GS)).astype(np.int64), 0, W - 1)
    by = np.clip(cy - 2, 0, H - 5)
    bx = np.clip(cx - 2, 0, W - 5)
    offs = np.arange(5, dtype=np.int64)
    iy = by[..., None] + offs          # (..., 5)
    ix = bx[..., None] + offs
    cyc = (f(GS) * iy + f(GS / 2.0)).astype(f)
    cxc = (f(GS) * ix + f(GS / 2.0)).astype(f)
    dy = y[..., None] - cyc
    dx = x[..., None] - cxc
    d2 = (dy * dy)[..., :, None] + (dx * dx)[..., None, :]   # (..., 5, 5)
    ids = iy[..., :, None] * W + ix[..., None, :]
    d2 = d2.reshape(d2.shape[:-2] + (25,))
    ids = ids.reshape(ids.shape[:-2] + (25,))
    # candidates are flat-id ascending, so a stable sort on d2 reproduces
    # top_k's lower-index tie-break
    order = np.argsort(d2, axis=-1, kind="stable")[..., :4]
    return np.take_along_axis(ids, order, axis=-1)


def _warp(p, Hm):
    f = np.float32
    xy = p[..., ::-1]
    ph = np.concatenate([xy, np.ones_like(xy[..., :1])], axis=-1)
    wp = np.einsum("bij,bmj->bmi", Hm, ph).astype(f)
    wp = wp[..., :2] / (wp[..., 2:3] + f(1e-8))
    return wp[..., ::-1].astype(f)


def _centers(ids):
    f = np.float32
    yy = (ids // W).astype(f) * f(GS) + f(GS / 2.0)
    xx = (ids % W).astype(f) * f(GS) + f(GS / 2.0)
    return np.stack([yy, xx], axis=-1)


def _smallest8_ids(x):
    """Indices of the 8 smallest values per row, lax.top_k tie semantics
    (ties -> lower index). x: (N, M) -> (N, 8)."""
    return np.argsort(x, axis=-1, kind="stable")[:, :SOS_NEG]


def kernel(kp1, w_kp1, kp1_desc, desc2, homo12):
    global LAST_RESULTS
    import os

    f = np.float32
    kp1 = np.asarray(kp1, f)
    w_kp1 = np.asarray(w_kp1, f)
    kp1_desc = np.asarray(kp1_desc, f)
    desc2 = np.asarray(desc2, f)
    homo12 = np.asarray(homo12, f)

    # ---------------- host geometry / small tensors ----------------
    w_kp1_desc = _sample_descriptors(desc2, w_kp1)                  # (B,N,C)
    pos = f(2.0) - f(2.0) * np.einsum("bnc,bnc->bn", kp1_desc, w_kp1_desc)

    cell4 = _nearest4(kp1)                                          # (B,N,4)
    kp1_cells = _centers(cell4.reshape(B, 4 * N))                   # (B,4N,2)
    warped = _warp(kp1_cells, homo12)                               # (B,4N,2)
    wcc = _nearest4(warped)                                         # (B,4N,4)
    ids16 = wcc.reshape(B, N, 16)                                   # neigh cells
    cell4_w = _nearest4(w_kp1)                                      # (B,N,4)

    # kp1_mask[n,n'] = #coinciding cells between cell4[n] and cell4[n']
    eqk = cell4[:, :, :, None, None] == cell4[:, None, None, :, :]
    kp1_mask = eqk.sum(axis=(2, 4)).astype(f)                       # (B,N,N)
    # w_kp1_mask[n,n'] = #coincidences between ids16[n] and cell4_w[n']
    eqw = ids16[:, :, :, None, None] == cell4_w[:, None, None, :, :]
    w_kp1_mask = eqw.sum(axis=(2, 4)).astype(f)                     # (B,N,N)

    # ---------------- device run ----------------
    nc = _get_nc()
    desc2_flat = np.ascontiguousarray(desc2.reshape(B, C, HW))
    lhsT_q = [np.ascontiguousarray(kp1_desc[b].T.astype(BF)) for b in range(B)]
    in_maps = []
    for b in range(B):
        for s in range(NSHARD):
            m = {"lhsT": lhsT_q[b]}
            for i, (c0, npc, eng) in enumerate(RHS_PIECES):
                m[f"rhs{i}"] = np.ascontiguousarray(
                    desc2_flat[b][:, s * SHW + c0:s * SHW + c0 + npc].astype(NP8)
                )
            in_maps.append(m)
    want_trace = bool(int(os.environ.get("KT_TRACE", "0")))
    try:
        res = run_bass_kernel_spmd(
            nc, in_maps, core_ids=list(range(8)), trace=want_trace
        )
    except ModuleNotFoundError:
        res = run_bass_kernel_spmd(nc, in_maps, core_ids=list(range(8)), trace=False)
    LAST_RESULTS = res
    results = res.results

    # candidate values per row: NSHARD shards x len(CHUNKS) chunks x 8, f32
    nch = len(CHUNKS)
    cand_all = np.empty((B, N, NSHARD, nch, 8), f)
    for ci, (b, s) in enumerate((b, s) for b in range(B) for s in range(NSHARD)):
        cf = results[ci]["cand"]                        # (RT,128,nch*8) f32
        for t in range(RT):
            rows = slice(t * 128, (t + 1) * 128)
            for k in range(nch):
                cand_all[b, rows, s, k, :] = cf[t][:, k * 8:(k + 1) * 8]

    # ---------------- fos: merge per-shard candidates ----------------
    # exact (host) raw scores of masked cells, replicating the device's
    # bf16-weights x e4m3-moving matmul: f32 accumulation over cast operands
    lhq = np.ascontiguousarray(  # (B,N,C) f32 of bf16
        np.stack([lhsT_q[b].T.astype(f) for b in range(B)]))
    dq = desc2_flat.astype(NP8).astype(f)               # (B,C,HW)
    vm16 = np.empty((B, N, 16), f)
    for b in range(B):
        gath = dq[b][:, ids16[b].reshape(-1)].reshape(C, N, 16)
        vm16[b] = np.einsum("nc,cnk->nk", lhq[b], gath)

    # numerics guard: recompute one row on the host and compare each chunk's
    # maximum (the max always survives the device's max tree); if PE numerics
    # don't replicate the host quantization model, fall back to exact host
    # evaluation for every row (correct, just slower on the host)
    g_row = dq[0].T @ lhq[0, 0]                         # (HW,)
    replication_ok = True
    for s in range(NSHARD):
        for k, (c0, c1, kind) in enumerate(CHUNKS):
            seg = g_row[s * SHW + c0:s * SHW + c1]
            if kind == "bf16":
                seg = seg.astype(BF).astype(f)
            if abs(float(seg.max()) - float(cand_all[0, 0, s, k].max())) > 0.02:
                replication_ok = False

    # chunk id (shard, chunk) for every cell
    cell_chunk = np.empty(HW, np.int64)
    chunk_kind = []
    for s in range(NSHARD):
        for k, (c0, c1, kind) in enumerate(CHUNKS):
            cell_chunk[s * SHW + c0: s * SHW + c1] = s * nch + k
            chunk_kind.append(kind)

    flat = cand_all.reshape(B, N, NSHARD * nch, 8)
    chunk_min = flat[..., 7]                            # (B,N,nchunks)
    TOL_F = 1e-3
    TOL_B = 0.033                                       # ~1 ulp bf16 at |x|~4
    neg_scores = np.empty((B, N, NUM_NEG), f)
    if not replication_ok:
        repair = [(b, n) for b in range(B) for n in range(N)]
    else:
        repair = []
    for b in range(B):
        for n in range(N):
            if not replication_ok:
                continue
            vals = flat[b, n].copy()                    # (nchunks, 8)
            alive = np.ones_like(vals, bool)
            uq, cnts = np.unique(ids16[b, n], return_counts=True)
            bad = False
            add = np.empty(len(uq), f)
            for i, (u, cu) in enumerate(zip(uq, cnts)):
                # value of this masked cell under device convention
                j = int(np.argmax(ids16[b, n] == u))
                v = vm16[b, n, j]
                ch = cell_chunk[u]
                kind = chunk_kind[ch]
                vq = f(BF(v)) if kind == "bf16" else v
                add[i] = v - f(2.5) * cu
                tol = TOL_B if kind == "bf16" else TOL_F
                if vq >= chunk_min[b, n, ch] - tol:
                    row = vals[ch]
                    cand_idx = np.where(alive[ch])[0]
                    if len(cand_idx):
                        d = np.abs(row[cand_idx] - vq)
                        jj = int(np.argmin(d))
                        if d[jj] <= tol:
                            alive[ch, cand_idx[jj]] = False
                        elif kind == "f32":
                            bad = True  # should have been exported; wasn't
                    # bf16 chunks: no match => shadowed by tree, accept
            if not bad:
                pool = np.concatenate([vals[alive], add])
                pool.sort()
                top = pool[::-1][:NUM_NEG]
                thr = top[-1]
                # certificate: no chunk may conceal values above thr
                for ch in range(NSHARD * nch):
                    tol = TOL_B if chunk_kind[ch] == "bf16" else TOL_F
                    if chunk_min[b, n, ch] >= thr - tol:
                        bad = True
                        break
            if bad:
                repair.append((b, n))
            else:
                neg_scores[b, n] = top

    if repair:
        hwdesc = desc2_flat.transpose(0, 2, 1)          # (B,HW,C) f32 exact
        for b, n in repair:
            row = hwdesc[b] @ kp1_desc[b, n]            # (HW,)
            np.subtract.at(row, ids16[b, n], f(2.5))
            neg_scores[b, n] = np.sort(row)[::-1][:NUM_NEG]

    neg = f(2.0) - f(2.0) * neg_scores                  # (B,N,16)
    fos = np.mean(
        np.maximum(pos[..., None] - neg + f(MARGIN), f(0.0)) ** 2
    ).astype(f)

    # ---------------- sos (host: 512-wide sims are cheap) ----------------
    k_ids = np.empty((B, N, SOS_NEG), np.int64)
    w_ids = np.empty((B, N, SOS_NEG), np.int64)
    for b in range(B):
        ksim = f(2.0) - f(2.0) * (kp1_desc[b] @ kp1_desc[b].T) \
            + f(5.0) * kp1_mask[b]
        wsim = f(2.0) - f(2.0) * (w_kp1_desc[b] @ w_kp1_desc[b].T) \
            + f(5.0) * w_kp1_mask[b]
        k_ids[b] = _smallest8_ids(ksim)
        w_ids[b] = _smallest8_ids(wsim)

    kd = np.take_along_axis(
        kp1_desc, k_ids.reshape(B, N * SOS_NEG)[:, :, None], axis=1
    ).reshape(B, N, SOS_NEG, C)
    wd = np.take_along_axis(
        w_kp1_desc, w_ids.reshape(B, N * SOS_NEG)[:, :, None], axis=1
    ).reshape(B, N, SOS_NEG, C)
    a = f(2.0) - f(2.0) * np.einsum("bnc,bnkc->bnk", kp1_desc, kd)
    bb = f(2.0) - f(2.0) * np.einsum("bnc,bnkc->bnk", w_kp1_desc, wd)
    sv = (a - bb).astype(f)
    sos = np.mean(np.sqrt(np.sum(sv * sv, axis=-1))).astype(f)

    return np.asarray(fos + sos, dtype=np.float32)


# revision 38
# speedup vs baseline: 1.2157x; 1.0094x over previous
"""Trainium2 Bass kernel for nn_HardQuadTripletSOSRLoss.

Sharding: 8 cores = 2 batches x 4 HW-shards (4096 grid cells each).

Device (per core): dsim candidate extraction only.
  - inputs host-quantized: kp1_desc^T as bf16 stationary, desc2 shard as
    fp8(e4m3) moving operand (halves DMA; matmul runs at bf16 speed)
  - a few garbage warm-up matmuls release the PE HAM clock throttle before
    the real data lands; rhs arrives as two 2048-cell pieces on separate
    DMA queues (scalar HWDGE + gpsimd SWDGE)
  - PE: scores = kp1_desc[b] @ desc2f[b, shard]^T, 512-col matmuls into
    [128, 1024] PSUM tiles (2 banks x 4 slots)
  - per 4096-cell row-tile: DVE max8 over a 1024-wide direct f32 chunk;
    ACT converts the other 3072 cells to bf16 in SBUF and DVE runs a
    2x-mode pairwise-max tree + final max8 over them
  - exports top-8 VALUES per chunk only (no indices)

Host: bilinear sampling, grid geometry, masks, k_sim/w_sim top-8 (512-wide,
cheap), distributed top-k merge with mask patching by value-match against
host-requantized scores (remove matched raw values, insert exact adjusted
values) + a per-chunk certificate; rows whose certificate fails are
recomputed exactly, and a whole-row numerics guard falls back to exact
host evaluation if the device quantization model ever stops replicating.
"""

import numpy as np
import ml_dtypes

import concourse.bass as bass
import concourse.mybir as mybir
import concourse.tile as tile
from concourse import bacc
from concourse.bass_utils import run_bass_kernel_spmd

# ---- problem constants (hardcoded per contract) ----
B, N, C, H, W = 2, 512, 128, 128, 128
HW = H * W
GS = 8
NUM_NEG = 16
SOS_NEG = 8
MARGIN = 1.0
NSHARD = 4
SHW = HW // NSHARD          # 4096 cells per shard
RT = N // 128               # 4 row tiles

F32 = mybir.dt.float32
BF16 = mybir.dt.bfloat16
F8 = mybir.dt.float8e4
BF = ml_dtypes.bfloat16
NP8 = ml_dtypes.float8_e4m3

SCAN_MODE = "split"         # "direct" | "split"
N_WARM = 4                  # dummy matmuls to release the PE HAM throttle

# per-row-tile chunk layout within one 4096-cell shard:
#   (start_cell, end_cell, kind)  kind: "f32" (exact top8) | "bf16" (tree)
if SCAN_MODE == "direct":
    CHUNKS = [(0, 2048, "f32"), (2048, 4096, "f32")]
else:
    # converted chunk first: it consumes the earliest-arriving DMA piece,
    # and each row-tile's critical path ends on the cheap direct max8
    CHUNKS = [(0, 3072, "bf16"), (3072, 4096, "f32")]
NF32 = sum(1 for c in CHUNKS if c[2] == "f32")
NBF = sum(1 for c in CHUNKS if c[2] == "bf16")

# rhs DMA pieces: (start_cell, n_cells, issue_engine); 2048-cell pieces keep
# 2KB descriptor rows (fp8) for decent per-queue DMA bandwidth
RHS_PIECES = [
    (0, 2048, "scalar"),
    (2048, 2048, "gpsimd"),
]

_NC_CACHE = {}
LAST_RESULTS = None  # BassKernelResults of most recent device run (for test.py)


def _build_nc():
    nc = bacc.Bacc("TRN2", target_bir_lowering=False, debug=False, num_devices=8)

    lhsT = nc.dram_tensor("lhsT", [C, N], BF16, kind="ExternalInput")
    rhs_dram = [
        nc.dram_tensor(f"rhs{i}", [C, npc], F8, kind="ExternalInput")
        for i, (c0, npc, eng) in enumerate(RHS_PIECES)
    ]
    nch = len(CHUNKS)
    cand = nc.dram_tensor("cand", [RT, 128, nch * 8], F32, kind="ExternalOutput")

    with tile.TileContext(nc) as tc:
        with (
            tc.tile_pool(name="const", bufs=1) as cpool,
            tc.tile_pool(name="cnd", bufs=2) as cndpool,
            tc.tile_pool(name="tree", bufs=2) as trpool,
            tc.tile_pool(name="psum", bufs=4, space="PSUM") as pspool,
        ):
            # PE warm-up: garbage matmuls release the HAM clock throttle
            # (~3.4us of sustained activity) before the real data lands.
            warm_w = cpool.tile([C, 128], BF16, tag="warmw")
            warm_x = cpool.tile([C, 512], F8, tag="warmx")
            nc.vector.memset(warm_w[:], 0.0)
            nc.vector.memset(warm_x[:], 0.0)
            if N_WARM:
                wp = pspool.tile([128, 1024], F32, tag="ps")
                for _ in range(N_WARM):
                    nc.tensor.matmul(wp[:, 0:512], warm_w[:], warm_x[:],
                                     start=True, stop=True)

            lhsT_sb = cpool.tile([C, N], BF16, tag="lhsT")
            nc.sync.dma_start(lhsT_sb[:], lhsT[:, :])
            rhs_sb = []
            for i, (c0, npc, eng) in enumerate(RHS_PIECES):
                t = cpool.tile([C, npc], F8, tag=f"rhs{i}")
                getattr(nc, eng).dma_start(t[:], rhs_dram[i][:, :])
                rhs_sb.append(t)

            def weights(t):
                return lhsT_sb[:, t * 128:(t + 1) * 128]

            def mm(ps_slice, t, cell0):
                # 512-col matmul: scores for shard cells [cell0, cell0+512)
                for i, (c0, npc, eng) in enumerate(RHS_PIECES):
                    if c0 <= cell0 < c0 + npc:
                        piece, col = rhs_sb[i], cell0 - c0
                        break
                nc.tensor.matmul(ps_slice, weights(t), piece[:, col:col + 512],
                                 start=True, stop=True)

            if SCAN_MODE == "direct":
                for t in range(RT):
                    cn = cndpool.tile([128, nch * 8], F32, tag="cn")
                    for ci in range(2):
                        ps = pspool.tile([128, 2048], F32, tag="ps")
                        for k in range(4):
                            mm(ps[:, k * 512:(k + 1) * 512], t,
                               ci * 2048 + k * 512)
                        nc.vector.max(cn[:, ci * 8:(ci + 1) * 8], ps[:])
                    nc.sync.dma_start(cand[t], cn[:])
            else:
                # per row-tile: bf16 max tree (ACT converts, DVE 2x TT-max)
                # over cells [0:3072) plus a direct f32 max8 over [3072:4096)
                for t in range(RT):
                    cn = cndpool.tile([128, nch * 8], F32, tag="cn")
                    ps = {}
                    cv = {}
                    for q in (0, 1, 2, 3):
                        p = pspool.tile([128, 1024], F32, tag="ps")
                        mm(p[:, 0:512], t, q * 1024)
                        if t == 0 and q == 0:
                            # half-width converts so ACT starts after the
                            # very first matmul instead of the second
                            c = trpool.tile([128, 1024], BF16, tag=f"c{q}")
                            nc.scalar.copy(c[:, 0:512], p[:, 0:512])
                            mm(p[:, 512:1024], t, q * 1024 + 512)
                            nc.scalar.copy(c[:, 512:1024], p[:, 512:1024])
                            ps[q] = p
                            cv[q] = c
                            continue
                        mm(p[:, 512:1024], t, q * 1024 + 512)
                        ps[q] = p
                        if q < 3:
                            c = trpool.tile([128, 1024], BF16, tag=f"c{q}")
                            nc.scalar.copy(c[:], p[:])
                            cv[q] = c
                    m1 = trpool.tile([128, 1024], BF16, tag="m1")
                    nc.vector.tensor_max(m1[:], cv[0][:], cv[1][:])
                    m2 = trpool.tile([128, 1024], BF16, tag="m2")
                    nc.vector.tensor_max(m2[:], m1[:], cv[2][:])
                    m3 = trpool.tile([128, 512], BF16, tag="m3")
                    nc.vector.tensor_max(m3[:], m2[:, 0:512], m2[:, 512:1024])
                    # bf16 tree values written into the f32 cn tile
                    nc.vector.max(cn[:, 0:8], m3[:])
                    # direct chunk: cells [3072, 4096)
                    nc.vector.max(cn[:, 8:16], ps[3][:])
                    nc.sync.dma_start(cand[t], cn[:])

    nc.compile()
    return nc


def _get_nc():
    if "nc" not in _NC_CACHE:
        _NC_CACHE["nc"] = _build_nc()
    return _NC_CACHE["nc"]


# ---------------- host-side helpers (all float32, mirror reference) ----------


def _sample_descriptors(desc2, kp):
    """Bilinear sample of desc2 (B,C,H,W) at image-space (y,x) kp, L2-normed."""
    b, c, h, w = desc2.shape
    f = np.float32
    y = np.clip(kp[..., 0] / f(GS) - f(0.5), f(0.0), f(h - 1.0)).astype(f)
    x = np.clip(kp[..., 1] / f(GS) - f(0.5), f(0.0), f(w - 1.0)).astype(f)
    y0 = np.clip(np.floor(y), 0, h - 2).astype(np.int64)
    x0 = np.clip(np.floor(x), 0, w - 2).astype(np.int64)
    wy = (y - y0.astype(f))[..., None]
    wx = (x - x0.astype(f))[..., None]
    dmap = desc2.transpose(0, 2, 3, 1).reshape(b, h * w, c)

    def g(yi, xi):
        idx = yi * w + xi
        return np.take_along_axis(dmap, idx[..., None], axis=1)

    v = (
        g(y0, x0) * (1 - wy) * (1 - wx)
        + g(y0, x0 + 1) * (1 - wy) * wx
        + g(y0 + 1, x0) * wy * (1 - wx)
        + g(y0 + 1, x0 + 1) * wy * wx
    )
    n = np.sqrt(np.sum(v * v, axis=-1, keepdims=True)).astype(f)
    return (v / (n + f(1e-8))).astype(f)


def _nearest4(pts):
    """Flat ids (..., 4) of the 4 nearest grid-cell centers, matching the
    reference's top_k over all HW cells (ties -> lower flat id)."""
    f = np.float32
    y = pts[..., 0]
    x = pts[..., 1]
    cy = np.clip(np.floor(y / f(GS)).astype(np.int64), 0, H - 1)
    cx = np.clip(np.floor(x / f(GS)).astype(np.int64), 0, W - 1)
    by = np.clip(cy - 2, 0, H - 5)
    bx = np.clip(cx - 2, 0, W - 5)
    offs = np.arange(5, dtype=np.int64)
    iy = by[..., None] + offs          # (..., 5)
    ix = bx[..., None] + offs
    cyc = (f(GS) * iy + f(GS / 2.0)).astype(f)
    cxc = (f(GS) * ix + f(GS / 2.0)).astype(f)
    dy = y[..., None] - cyc
    dx = x[..., None] - cxc
    d2 = (dy * dy)[..., :, None] + (dx * dx)[..., None, :]   # (..., 5, 5)
    ids = iy[..., :, None] * W + ix[..., None, :]
    d2 = d2.reshape(d2.shape[:-2] + (25,))
    ids = ids.reshape(ids.shape[:-2] + (25,))
    # candidates are flat-id ascending, so a stable sort on d2 reproduces
    # top_k's lower-index tie-break
    order = np.argsort(d2, axis=-1, kind="stable")[..., :4]
    return np.take_along_axis(ids, order, axis=-1)


def _warp(p, Hm):
    f = np.float32
    xy = p[..., ::-1]
    ph = np.concatenate([xy, np.ones_like(xy[..., :1])], axis=-1)
    wp = np.einsum("bij,bmj->bmi", Hm, ph).astype(f)
    wp = wp[..., :2] / (wp[..., 2:3] + f(1e-8))
    return wp[..., ::-1].astype(f)


def _centers(ids):
    f = np.float32
    yy = (ids // W).astype(f) * f(GS) + f(GS / 2.0)
    xx = (ids % W).astype(f) * f(GS) + f(GS / 2.0)
    return np.stack([yy, xx], axis=-1)


def _smallest8_ids(x):
    """Indices of the 8 smallest values per row, lax.top_k tie semantics
    (ties -> lower index). x: (N, M) -> (N, 8)."""
    return np.argsort(x, axis=-1, kind="stable")[:, :SOS_NEG]


def kernel(kp1, w_kp1, kp1_desc, desc2, homo12):
    global LAST_RESULTS
    import os

    f = np.float32
    kp1 = np.asarray(kp1, f)
    w_kp1 = np.asarray(w_kp1, f)
    kp1_desc = np.asarray(kp1_desc, f)
    desc2 = np.asarray(desc2, f)
    homo12 = np.asarray(homo12, f)

    # ---------------- host geometry / small tensors ----------------
    w_kp1_desc = _sample_descriptors(desc2, w_kp1)                  # (B,N,C)
    pos = f(2.0) - f(2.0) * np.einsum("bnc,bnc->bn", kp1_desc, w_kp1_desc)

    cell4 = _nearest4(kp1)                                          # (B,N,4)
    kp1_cells = _centers(cell4.reshape(B, 4 * N))                   # (B,4N,2)
    warped = _warp(kp1_cells, homo12)                               # (B,4N,2)
    wcc = _nearest4(warped)                                         # (B,4N,4)
    ids16 = wcc.reshape(B, N, 16)                                   # neigh cells
    cell4_w = _nearest4(w_kp1)                                      # (B,N,4)

    # kp1_mask[n,n'] = #coinciding cells between cell4[n] and cell4[n']
    eqk = cell4[:, :, :, None, None] == cell4[:, None, None, :, :]
    kp1_mask = eqk.sum(axis=(2, 4)).astype(f)                       # (B,N,N)
    # w_kp1_mask[n,n'] = #coincidences between ids16[n] and cell4_w[n']
    eqw = ids16[:, :, :, None, None] == cell4_w[:, None, None, :, :]
    w_kp1_mask = eqw.sum(axis=(2, 4)).astype(f)                     # (B,N,N)

    # ---------------- device run ----------------
    nc = _get_nc()
    desc2_flat = np.ascontiguousarray(desc2.reshape(B, C, HW))
    lhsT_q = [np.ascontiguousarray(kp1_desc[b].T.astype(BF)) for b in range(B)]
    in_maps = []
    for b in range(B):
        for s in range(NSHARD):
            m = {"lhsT": lhsT_q[b]}
            for i, (c0, npc, eng) in enumerate(RHS_PIECES):
                m[f"rhs{i}"] = np.ascontiguousarray(
                    desc2_flat[b][:, s * SHW + c0:s * SHW + c0 + npc].astype(NP8)
                )
            in_maps.append(m)
    want_trace = bool(int(os.environ.get("KT_TRACE", "0")))
    try:
        res = run_bass_kernel_spmd(
            nc, in_maps, core_ids=list(range(8)), trace=want_trace
        )
    except ModuleNotFoundError:
        res = run_bass_kernel_spmd(nc, in_maps, core_ids=list(range(8)), trace=False)
    LAST_RESULTS = res
    results = res.results

    # candidate values per row: NSHARD shards x len(CHUNKS) chunks x 8, f32
    nch = len(CHUNKS)
    cand_all = np.empty((B, N, NSHARD, nch, 8), f)
    for ci, (b, s) in enumerate((b, s) for b in range(B) for s in range(NSHARD)):
        cf = results[ci]["cand"]                        # (RT,128,nch*8) f32
        for t in range(RT):
            rows = slice(t * 128, (t + 1) * 128)
            for k in range(nch):
                cand_all[b, rows, s, k, :] = cf[t][:, k * 8:(k + 1) * 8]

    # ---------------- fos: merge per-shard candidates ----------------
    # exact (host) raw scores of masked cells, replicating the device's
    # bf16-weights x e4m3-moving matmul: f32 accumulation over cast operands
    lhq = np.ascontiguousarray(  # (B,N,C) f32 of bf16
        np.stack([lhsT_q[b].T.astype(f) for b in range(B)]))
    dq = desc2_flat.astype(NP8).astype(f)               # (B,C,HW)
    vm16 = np.empty((B, N, 16), f)
    for b in range(B):
        gath = dq[b][:, ids16[b].reshape(-1)].reshape(C, N, 16)
        vm16[b] = np.einsum("nc,cnk->nk", lhq[b], gath)

    # numerics guard: recompute one row on the host and compare each chunk's
    # maximum (the max always survives the device's max tree); if PE numerics
    # don't replicate the host quantization model, fall back to exact host
    # evaluation for every row (correct, just slower on the host)
    g_row = dq[0].T @ lhq[0, 0]                         # (HW,)
    replication_ok = True
    for s in range(NSHARD):
        for k, (c0, c1, kind) in enumerate(CHUNKS):
            seg = g_row[s * SHW + c0:s * SHW + c1]
            if kind == "bf16":
                seg = seg.astype(BF).astype(f)
            if abs(float(seg.max()) - float(cand_all[0, 0, s, k].max())) > 0.02:
                replication_ok = False

    # chunk id (shard, chunk) for every cell
    cell_chunk = np.empty(HW, np.int64)
    chunk_kind = []
    for s in range(NSHARD):
        for k, (c0, c1, kind) in enumerate(CHUNKS):
            cell_chunk[s * SHW + c0: s * SHW + c1] = s * nch + k
            chunk_kind.append(kind)

    flat = cand_all.reshape(B, N, NSHARD * nch, 8)
    chunk_min = flat[..., 7]                            # (B,N,nchunks)
    TOL_F = 1e-3
    TOL_B = 0.033                                       # ~1 ulp bf16 at |x|~4
    neg_scores = np.empty((B, N, NUM_NEG), f)
    if not replication_ok:
        repair = [(b, n) for b in range(B) for n in range(N)]
    else:
        repair = []
    for b in range(B):
        for n in range(N):
            if not replication_ok:
                continue
            vals = flat[b, n].copy()                    # (nchunks, 8)
            alive = np.ones_like(vals, bool)
            uq, cnts = np.unique(ids16[b, n], return_counts=True)
            bad = False
            add = np.empty(len(uq), f)
            for i, (u, cu) in enumerate(zip(uq, cnts)):
                # value of this masked cell under device convention
                j = int(np.argmax(ids16[b, n] == u))
                v = vm16[b, n, j]
                ch = cell_chunk[u]
                kind = chunk_kind[ch]
                vq = f(BF(v)) if kind == "bf16" else v
                add[i] = v - f(2.5) * cu
                tol = TOL_B if kind == "bf16" else TOL_F
                if vq >= chunk_min[b, n, ch] - tol:
                    row = vals[ch]
                    cand_idx = np.where(alive[ch])[0]
                    if len(cand_idx):
                        d = np.abs(row[cand_idx] - vq)
                        jj = int(np.argmin(d))
                        if d[jj] <= tol:
                            alive[ch, cand_idx[jj]] = False
                        elif kind == "f32":
                            bad = True  # should have been exported; wasn't
                    # bf16 chunks: no match => shadowed by tree, accept
            if not bad:
                pool = np.concatenate([vals[alive], add])
                pool.sort()
                top = pool[::-1][:NUM_NEG]
                thr = top[-1]
                # certificate: no chunk may conceal values above thr
                for ch in range(NSHARD * nch):
                    tol = TOL_B if chunk_kind[ch] == "bf16" else TOL_F
                    if chunk_min[b, n, ch] >= thr - tol:
                        bad = True
                        break
            if bad:
                repair.append((b, n))
            else:
                neg_scores[b, n] = top

    if repair:
        hwdesc = desc2_flat.transpose(0, 2, 1)          # (B,HW,C) f32 exact
        for b, n in repair:
            row = hwdesc[b] @ kp1_desc[b, n]            # (HW,)
            np.subtract.at(row, ids16[b, n], f(2.5))
            neg_scores[b, n] = np.sort(row)[::-1][:NUM_NEG]

    neg = f(2.0) - f(2.0) * neg_scores                  # (B,N,16)
    fos = np.mean(
        np.maximum(pos[..., None] - neg + f(MARGIN), f(0.0)) ** 2
    ).astype(f)

    # ---------------- sos (host: 512-wide sims are cheap) ----------------
    k_ids = np.empty((B, N, SOS_NEG), np.int64)
    w_ids = np.empty((B, N, SOS_NEG), np.int64)
    for b in range(B):
        ksim = f(2.0) - f(2.0) * (kp1_desc[b] @ kp1_desc[b].T) \
            + f(5.0) * kp1_mask[b]
        wsim = f(2.0) - f(2.0) * (w_kp1_desc[b] @ w_kp1_desc[b].T) \
            + f(5.0) * w_kp1_mask[b]
        k_ids[b] = _smallest8_ids(ksim)
        w_ids[b] = _smallest8_ids(wsim)

    kd = np.take_along_axis(
        kp1_desc, k_ids.reshape(B, N * SOS_NEG)[:, :, None], axis=1
    ).reshape(B, N, SOS_NEG, C)
    wd = np.take_along_axis(
        w_kp1_desc, w_ids.reshape(B, N * SOS_NEG)[:, :, None], axis=1
    ).reshape(B, N, SOS_NEG, C)
    a = f(2.0) - f(2.0) * np.einsum("bnc,bnkc->bnk", kp1_desc, kd)
    bb = f(2.0) - f(2.0) * np.einsum("bnc,bnkc->bnk", w_kp1_desc, wd)
    sv = (a - bb).astype(f)
    sos = np.mean(np.sqrt(np.sum(sv * sv, axis=-1))).astype(f)

    return np.asarray(fos + sos, dtype=np.float32)


# revision 39
# speedup vs baseline: 1.2415x; 1.0212x over previous
"""Trainium2 Bass kernel for nn_HardQuadTripletSOSRLoss.

Sharding: 8 cores = 2 batches x 4 HW-shards (4096 grid cells each).

Device (per core): dsim candidate extraction only.
  - inputs host-quantized: kp1_desc^T as bf16 stationary, desc2 shard as
    fp8(e4m3) moving operand (halves DMA; matmul runs at bf16 speed)
  - a few garbage warm-up matmuls release the PE HAM clock throttle before
    the real data lands; rhs arrives as two 2048-cell pieces on separate
    DMA queues (scalar HWDGE + gpsimd SWDGE)
  - PE: scores = kp1_desc[b] @ desc2f[b, shard]^T, 512-col matmuls into
    [128, 1024] PSUM tiles (2 banks x 4 slots)
  - per 4096-cell row-tile: DVE max8 over a 1024-wide direct f32 chunk;
    ACT converts the other 3072 cells to bf16 in SBUF and DVE runs a
    2x-mode pairwise-max tree + final max8 over them
  - exports top-8 VALUES per chunk only (no indices)

Host: bilinear sampling, grid geometry, masks, k_sim/w_sim top-8 (512-wide,
cheap), distributed top-k merge with mask patching by value-match against
host-requantized scores (remove matched raw values, insert exact adjusted
values) + a per-chunk certificate; rows whose certificate fails are
recomputed exactly, and a whole-row numerics guard falls back to exact
host evaluation if the device quantization model ever stops replicating.
"""

import numpy as np
import ml_dtypes

import concourse.bass as bass
import concourse.mybir as mybir
import concourse.tile as tile
from concourse import bacc
from concourse.bass_utils import run_bass_kernel_spmd

# ---- problem constants (hardcoded per contract) ----
B, N, C, H, W = 2, 512, 128, 128, 128
HW = H * W
GS = 8
NUM_NEG = 16
SOS_NEG = 8
MARGIN = 1.0
NSHARD = 4
SHW = HW // NSHARD          # 4096 cells per shard
RT = N // 128               # 4 row tiles

F32 = mybir.dt.float32
BF16 = mybir.dt.bfloat16
F8 = mybir.dt.float8e4
BF = ml_dtypes.bfloat16
NP8 = ml_dtypes.float8_e4m3

SCAN_MODE = "split"         # "direct" | "split"
N_WARM = 4                  # dummy matmuls to release the PE HAM throttle

# per-row-tile chunk layout within one 4096-cell shard:
#   (start_cell, end_cell, kind)  kind: "f32" (exact top8) | "bf16" (tree)
if SCAN_MODE == "direct":
    CHUNKS = [(0, 2048, "f32"), (2048, 4096, "f32")]
else:
    # converted chunk first: it consumes the earliest-arriving DMA piece,
    # and each row-tile's critical path ends on the cheap direct max8
    CHUNKS = [(0, 3072, "bf16"), (3072, 4096, "f32")]
NF32 = sum(1 for c in CHUNKS if c[2] == "f32")
NBF = sum(1 for c in CHUNKS if c[2] == "bf16")

# rhs DMA pieces: (start_cell, n_cells, issue_engine); 2048-cell pieces keep
# 2KB descriptor rows (fp8) for decent per-queue DMA bandwidth
RHS_PIECES = [
    (0, 2048, "scalar"),
    (2048, 2048, "gpsimd"),
]

_NC_CACHE = {}
LAST_RESULTS = None  # BassKernelResults of most recent device run (for test.py)


def _build_nc():
    nc = bacc.Bacc("TRN2", target_bir_lowering=False, debug=False, num_devices=8)

    lhsT = nc.dram_tensor("lhsT", [C, N], BF16, kind="ExternalInput")
    rhs_dram = [
        nc.dram_tensor(f"rhs{i}", [C, npc], F8, kind="ExternalInput")
        for i, (c0, npc, eng) in enumerate(RHS_PIECES)
    ]
    nch = len(CHUNKS)
    cand = nc.dram_tensor("cand", [RT, 128, nch * 8], F32, kind="ExternalOutput")

    with tile.TileContext(nc) as tc:
        with (
            tc.tile_pool(name="const", bufs=1) as cpool,
            tc.tile_pool(name="cnd", bufs=2) as cndpool,
            tc.tile_pool(name="tree", bufs=2) as trpool,
            tc.tile_pool(name="psum", bufs=4, space="PSUM") as pspool,
        ):
            # PE warm-up: garbage matmuls release the HAM clock throttle
            # (~3.4us of sustained activity) before the real data lands.
            warm_w = cpool.tile([C, 128], BF16, tag="warmw")
            warm_x = cpool.tile([C, 512], F8, tag="warmx")
            nc.vector.memset(warm_w[:], 0.0)
            nc.vector.memset(warm_x[:], 0.0)
            if N_WARM:
                wp = pspool.tile([128, 1024], F32, tag="ps")
                for _ in range(N_WARM):
                    nc.tensor.matmul(wp[:, 0:512], warm_w[:], warm_x[:],
                                     start=True, stop=True)

            lhsT_sb = cpool.tile([C, N], BF16, tag="lhsT")
            nc.sync.dma_start(lhsT_sb[:], lhsT[:, :])
            rhs_sb = []
            for i, (c0, npc, eng) in enumerate(RHS_PIECES):
                t = cpool.tile([C, npc], F8, tag=f"rhs{i}")
                getattr(nc, eng).dma_start(t[:], rhs_dram[i][:, :])
                rhs_sb.append(t)

            def weights(t):
                return lhsT_sb[:, t * 128:(t + 1) * 128]

            def mm(ps_slice, t, cell0):
                # 512-col matmul: scores for shard cells [cell0, cell0+512)
                for i, (c0, npc, eng) in enumerate(RHS_PIECES):
                    if c0 <= cell0 < c0 + npc:
                        piece, col = rhs_sb[i], cell0 - c0
                        break
                nc.tensor.matmul(ps_slice, weights(t), piece[:, col:col + 512],
                                 start=True, stop=True)

            if SCAN_MODE == "direct":
                for t in range(RT):
                    cn = cndpool.tile([128, nch * 8], F32, tag="cn")
                    for ci in range(2):
                        ps = pspool.tile([128, 2048], F32, tag="ps")
                        for k in range(4):
                            mm(ps[:, k * 512:(k + 1) * 512], t,
                               ci * 2048 + k * 512)
                        nc.vector.max(cn[:, ci * 8:(ci + 1) * 8], ps[:])
                    nc.sync.dma_start(cand[t], cn[:])
            else:
                # per row-tile: bf16 max tree (ACT converts, DVE 2x TT-max)
                # over cells [0:3072) plus a direct f32 max8 over [3072:4096)
                for t in range(RT):
                    cn = cndpool.tile([128, nch * 8], F32, tag="cn")
                    ps = {}
                    cv = {}
                    for q in (0, 1, 2, 3):
                        p = pspool.tile([128, 1024], F32, tag="ps")
                        mm(p[:, 0:512], t, q * 1024)
                        if t == 0 and q == 0:
                            # half-width converts so ACT starts after the
                            # very first matmul instead of the second
                            c = trpool.tile([128, 1024], BF16, tag=f"c{q}")
                            nc.scalar.copy(c[:, 0:512], p[:, 0:512])
                            mm(p[:, 512:1024], t, q * 1024 + 512)
                            nc.scalar.copy(c[:, 512:1024], p[:, 512:1024])
                            ps[q] = p
                            cv[q] = c
                            continue
                        mm(p[:, 512:1024], t, q * 1024 + 512)
                        ps[q] = p
                        if q < 3:
                            c = trpool.tile([128, 1024], BF16, tag=f"c{q}")
                            nc.scalar.copy(c[:], p[:])
                            cv[q] = c
                    # direct chunk first in program order so the scheduler
                    # runs it as soon as its PSUM lands, keeping the slow
                    # [1024] f32 max8 off the tail of the tree chain
                    nc.vector.max(cn[:, 8:16], ps[3][:])
                    m1 = trpool.tile([128, 1024], BF16, tag="m1")
                    nc.vector.tensor_max(m1[:], cv[0][:], cv[1][:])
                    m2 = trpool.tile([128, 1024], BF16, tag="m2")
                    nc.vector.tensor_max(m2[:], m1[:], cv[2][:])
                    m3 = trpool.tile([128, 512], BF16, tag="m3")
                    nc.vector.tensor_max(m3[:], m2[:, 0:512], m2[:, 512:1024])
                    # bf16 tree values written into the f32 cn tile
                    nc.vector.max(cn[:, 0:8], m3[:])
                    nc.sync.dma_start(cand[t], cn[:])

    nc.compile()
    return nc


def _get_nc():
    if "nc" not in _NC_CACHE:
        _NC_CACHE["nc"] = _build_nc()
    return _NC_CACHE["nc"]


# ---------------- host-side helpers (all float32, mirror reference) ----------


def _sample_descriptors(desc2, kp):
    """Bilinear sample of desc2 (B,C,H,W) at image-space (y,x) kp, L2-normed."""
    b, c, h, w = desc2.shape
    f = np.float32
    y = np.clip(kp[..., 0] / f(GS) - f(0.5), f(0.0), f(h - 1.0)).astype(f)
    x = np.clip(kp[..., 1] / f(GS) - f(0.5), f(0.0), f(w - 1.0)).astype(f)
    y0 = np.clip(np.floor(y), 0, h - 2).astype(np.int64)
    x0 = np.clip(np.floor(x), 0, w - 2).astype(np.int64)
    wy = (y - y0.astype(f))[..., None]
    wx = (x - x0.astype(f))[..., None]
    dmap = desc2.transpose(0, 2, 3, 1).reshape(b, h * w, c)

    def g(yi, xi):
        idx = yi * w + xi
        return np.take_along_axis(dmap, idx[..., None], axis=1)

    v = (
        g(y0, x0) * (1 - wy) * (1 - wx)
        + g(y0, x0 + 1) * (1 - wy) * wx
        + g(y0 + 1, x0) * wy * (1 - wx)
        + g(y0 + 1, x0 + 1) * wy * wx
    )
    n = np.sqrt(np.sum(v * v, axis=-1, keepdims=True)).astype(f)
    return (v / (n + f(1e-8))).astype(f)


def _nearest4(pts):
    """Flat ids (..., 4) of the 4 nearest grid-cell centers, matching the
    reference's top_k over all HW cells (ties -> lower flat id)."""
    f = np.float32
    y = pts[..., 0]
    x = pts[..., 1]
    cy = np.clip(np.floor(y / f(GS)).astype(np.int64), 0, H - 1)
    cx = np.clip(np.floor(x / f(GS)).astype(np.int64), 0, W - 1)
    by = np.clip(cy - 2, 0, H - 5)
    bx = np.clip(cx - 2, 0, W - 5)
    offs = np.arange(5, dtype=np.int64)
    iy = by[..., None] + offs          # (..., 5)
    ix = bx[..., None] + offs
    cyc = (f(GS) * iy + f(GS / 2.0)).astype(f)
    cxc = (f(GS) * ix + f(GS / 2.0)).astype(f)
    dy = y[..., None] - cyc
    dx = x[..., None] - cxc
    d2 = (dy * dy)[..., :, None] + (dx * dx)[..., None, :]   # (..., 5, 5)
    ids = iy[..., :, None] * W + ix[..., None, :]
    d2 = d2.reshape(d2.shape[:-2] + (25,))
    ids = ids.reshape(ids.shape[:-2] + (25,))
    # candidates are flat-id ascending, so a stable sort on d2 reproduces
    # top_k's lower-index tie-break
    order = np.argsort(d2, axis=-1, kind="stable")[..., :4]
    return np.take_along_axis(ids, order, axis=-1)


def _warp(p, Hm):
    f = np.float32
    xy = p[..., ::-1]
    ph = np.concatenate([xy, np.ones_like(xy[..., :1])], axis=-1)
    wp = np.einsum("bij,bmj->bmi", Hm, ph).astype(f)
    wp = wp[..., :2] / (wp[..., 2:3] + f(1e-8))
    return wp[..., ::-1].astype(f)


def _centers(ids):
    f = np.float32
    yy = (ids // W).astype(f) * f(GS) + f(GS / 2.0)
    xx = (ids % W).astype(f) * f(GS) + f(GS / 2.0)
    return np.stack([yy, xx], axis=-1)


def _smallest8_ids(x):
    """Indices of the 8 smallest values per row, lax.top_k tie semantics
    (ties -> lower index). x: (N, M) -> (N, 8)."""
    return np.argsort(x, axis=-1, kind="stable")[:, :SOS_NEG]


def kernel(kp1, w_kp1, kp1_desc, desc2, homo12):
    global LAST_RESULTS
    import os

    f = np.float32
    kp1 = np.asarray(kp1, f)
    w_kp1 = np.asarray(w_kp1, f)
    kp1_desc = np.asarray(kp1_desc, f)
    desc2 = np.asarray(desc2, f)
    homo12 = np.asarray(homo12, f)

    # ---------------- host geometry / small tensors ----------------
    w_kp1_desc = _sample_descriptors(desc2, w_kp1)                  # (B,N,C)
    pos = f(2.0) - f(2.0) * np.einsum("bnc,bnc->bn", kp1_desc, w_kp1_desc)

    cell4 = _nearest4(kp1)                                          # (B,N,4)
    kp1_cells = _centers(cell4.reshape(B, 4 * N))                   # (B,4N,2)
    warped = _warp(kp1_cells, homo12)                               # (B,4N,2)
    wcc = _nearest4(warped)                                         # (B,4N,4)
    ids16 = wcc.reshape(B, N, 16)                                   # neigh cells
    cell4_w = _nearest4(w_kp1)                                      # (B,N,4)

    # kp1_mask[n,n'] = #coinciding cells between cell4[n] and cell4[n']
    eqk = cell4[:, :, :, None, None] == cell4[:, None, None, :, :]
    kp1_mask = eqk.sum(axis=(2, 4)).astype(f)                       # (B,N,N)
    # w_kp1_mask[n,n'] = #coincidences between ids16[n] and cell4_w[n']
    eqw = ids16[:, :, :, None, None] == cell4_w[:, None, None, :, :]
    w_kp1_mask = eqw.sum(axis=(2, 4)).astype(f)                     # (B,N,N)

    # ---------------- device run ----------------
    nc = _get_nc()
    desc2_flat = np.ascontiguousarray(desc2.reshape(B, C, HW))
    lhsT_q = [np.ascontiguousarray(kp1_desc[b].T.astype(BF)) for b in range(B)]
    in_maps = []
    for b in range(B):
        for s in range(NSHARD):
            m = {"lhsT": lhsT_q[b]}
            for i, (c0, npc, eng) in enumerate(RHS_PIECES):
                m[f"rhs{i}"] = np.ascontiguousarray(
                    desc2_flat[b][:, s * SHW + c0:s * SHW + c0 + npc].astype(NP8)
                )
            in_maps.append(m)
    want_trace = bool(int(os.environ.get("KT_TRACE", "0")))
    try:
        res = run_bass_kernel_spmd(
            nc, in_maps, core_ids=list(range(8)), trace=want_trace
        )
    except ModuleNotFoundError:
        res = run_bass_kernel_spmd(nc, in_maps, core_ids=list(range(8)), trace=False)
    LAST_RESULTS = res
    results = res.results

    # candidate values per row: NSHARD shards x len(CHUNKS) chunks x 8, f32
    nch = len(CHUNKS)
    cand_all = np.empty((B, N, NSHARD, nch, 8), f)
    for ci, (b, s) in enumerate((b, s) for b in range(B) for s in range(NSHARD)):
        cf = results[ci]["cand"]                        # (RT,128,nch*8) f32
        for t in range(RT):
            rows = slice(t * 128, (t + 1) * 128)
            for k in range(nch):
                cand_all[b, rows, s, k, :] = cf[t][:, k * 8:(k + 1) * 8]

    # ---------------- fos: merge per-shard candidates ----------------
    # exact (host) raw scores of masked cells, replicating the device's
    # bf16-weights x e4m3-moving matmul: f32 accumulation over cast operands
    lhq = np.ascontiguousarray(  # (B,N,C) f32 of bf16
        np.stack([lhsT_q[b].T.astype(f) for b in range(B)]))
    dq = desc2_flat.astype(NP8).astype(f)               # (B,C,HW)
    vm16 = np.empty((B, N, 16), f)
    for b in range(B):
        gath = dq[b][:, ids16[b].reshape(-1)].reshape(C, N, 16)
        vm16[b] = np.einsum("nc,cnk->nk", lhq[b], gath)

    # numerics guard: recompute one row on the host and compare each chunk's
    # maximum (the max always survives the device's max tree); if PE numerics
    # don't replicate the host quantization model, fall back to exact host
    # evaluation for every row (correct, just slower on the host)
    g_row = dq[0].T @ lhq[0, 0]                         # (HW,)
    replication_ok = True
    for s in range(NSHARD):
        for k, (c0, c1, kind) in enumerate(CHUNKS):
            seg = g_row[s * SHW + c0:s * SHW + c1]
            if kind == "bf16":
                seg = seg.astype(BF).astype(f)
            if abs(float(seg.max()) - float(cand_all[0, 0, s, k].max())) > 0.02:
                replication_ok = False

    # chunk id (shard, chunk) for every cell
    cell_chunk = np.empty(HW, np.int64)
    chunk_kind = []
    for s in range(NSHARD):
        for k, (c0, c1, kind) in enumerate(CHUNKS):
            cell_chunk[s * SHW + c0: s * SHW + c1] = s * nch + k
            chunk_kind.append(kind)

    flat = cand_all.reshape(B, N, NSHARD * nch, 8)
    chunk_min = flat[..., 7]                            # (B,N,nchunks)
    TOL_F = 1e-3
    TOL_B = 0.033                                       # ~1 ulp bf16 at |x|~4
    neg_scores = np.empty((B, N, NUM_NEG), f)
    if not replication_ok:
        repair = [(b, n) for b in range(B) for n in range(N)]
    else:
        repair = []
    for b in range(B):
        for n in range(N):
            if not replication_ok:
                continue
            vals = flat[b, n].copy()                    # (nchunks, 8)
            alive = np.ones_like(vals, bool)
            uq, cnts = np.unique(ids16[b, n], return_counts=True)
            bad = False
            add = np.empty(len(uq), f)
            for i, (u, cu) in enumerate(zip(uq, cnts)):
                # value of this masked cell under device convention
                j = int(np.argmax(ids16[b, n] == u))
                v = vm16[b, n, j]
                ch = cell_chunk[u]
                kind = chunk_kind[ch]
                vq = f(BF(v)) if kind == "bf16" else v
                add[i] = v - f(2.5) * cu
                tol = TOL_B if kind == "bf16" else TOL_F
                if vq >= chunk_min[b, n, ch] - tol:
                    row = vals[ch]
                    cand_idx = np.where(alive[ch])[0]
                    if len(cand_idx):
                        d = np.abs(row[cand_idx] - vq)
                        jj = int(np.argmin(d))
                        if d[jj] <= tol:
                            alive[ch, cand_idx[jj]] = False
                        elif kind == "f32":
                            bad = True  # should have been exported; wasn't
                    # bf16 chunks: no match => shadowed by tree, accept
            if not bad:
                pool = np.concatenate([vals[alive], add])
                pool.sort()
                top = pool[::-1][:NUM_NEG]
                thr = top[-1]
                # certificate: no chunk may conceal values above thr
                for ch in range(NSHARD * nch):
                    tol = TOL_B if chunk_kind[ch] == "bf16" else TOL_F
                    if chunk_min[b, n, ch] >= thr - tol:
                        bad = True
                        break
            if bad:
                repair.append((b, n))
            else:
                neg_scores[b, n] = top

    if repair:
        hwdesc = desc2_flat.transpose(0, 2, 1)          # (B,HW,C) f32 exact
        for b, n in repair:
            row = hwdesc[b] @ kp1_desc[b, n]            # (HW,)
            np.subtract.at(row, ids16[b, n], f(2.5))
            neg_scores[b, n] = np.sort(row)[::-1][:NUM_NEG]

    neg = f(2.0) - f(2.0) * neg_scores                  # (B,N,16)
    fos = np.mean(
        np.maximum(pos[..., None] - neg + f(MARGIN), f(0.0)) ** 2
    ).astype(f)

    # ---------------- sos (host: 512-wide sims are cheap) ----------------
    k_ids = np.empty((B, N, SOS_NEG), np.int64)
    w_ids = np.empty((B, N, SOS_NEG), np.int64)
    for b in range(B):
        ksim = f(2.0) - f(2.0) * (kp1_desc[b] @ kp1_desc[b].T) \
            + f(5.0) * kp1_mask[b]
        wsim = f(2.0) - f(2.0) * (w_kp1_desc[b] @ w_kp1_desc[b].T) \
            + f(5.0) * w_kp1_mask[b]
        k_ids[b] = _smallest8_ids(ksim)
        w_ids[b] = _smallest8_ids(wsim)

    kd = np.take_along_axis(
        kp1_desc, k_ids.reshape(B, N * SOS_NEG)[:, :, None], axis=1
    ).reshape(B, N, SOS_NEG, C)
    wd = np.take_along_axis(
        w_kp1_desc, w_ids.reshape(B, N * SOS_NEG)[:, :, None], axis=1
    ).reshape(B, N, SOS_NEG, C)
    a = f(2.0) - f(2.0) * np.einsum("bnc,bnkc->bnk", kp1_desc, kd)
    bb = f(2.0) - f(2.0) * np.einsum("bnc,bnkc->bnk", w_kp1_desc, wd)
    sv = (a - bb).astype(f)
    sos = np.mean(np.sqrt(np.sum(sv * sv, axis=-1))).astype(f)

    return np.asarray(fos + sos, dtype=np.float32)
